# revision 13
# baseline (speedup 1.0000x reference)
"""Trainium2 Bass kernel for the AOD dense-CNN dehazing network.

Data-parallel across 8 NeuronCores: one (3,512,512) image per core.
Per core, the 20-conv dense network runs as a sequence of banded bf16
matmuls on the TensorEngine:

  - Activations live in DRAM "planes" [20, 3, 548, 518] bf16, zero-padded
    3 rows/cols on each side (plane 0 = input x, planes 1..19 = x1..x19).
  - For each conv layer j and 36-row output block b, one DMA gathers the
    42-row input windows of all input planes into an SBUF "shingle"
    [126 = (3ch x 42rows), T*518] (T = cin/3 input planes).
  - The conv is computed as k*T accumulating matmuls: contraction over
    (channel, row-window) on partitions; kernel-x taps are free-dim
    shifts of the shingle; output M = (3 co x 36 rows) = 108.
  - ScalarE applies bias+ReLU while evacuating PSUM -> SBUF (bf16), and
    the result is DMA'd back to the layer's DRAM plane.
  - x20 is kept in SBUF; VectorE/ScalarE compute relu(x20*x - x20 + 1).

Raw Bass (no Tile framework): each engine runs a fully unrolled static
program; cross-engine sync uses single-wait semaphores with statically
computed thresholds.
"""

import numpy as np

import concourse.bass as bass
import concourse.mybir as mybir
from concourse.bass_utils import run_bass_kernel_spmd

try:
    import ml_dtypes

    _BF16 = ml_dtypes.bfloat16
except ImportError:  # pragma: no cover
    _BF16 = np.float32

# (kernel_size, cin) for conv1..conv20; all have 3 output channels
_CONV_SPECS = [
    (1, 3), (3, 3), (3, 6), (5, 6), (5, 6), (7, 9), (7, 12), (5, 12), (5, 15),
    (3, 15), (3, 18), (3, 18), (3, 21), (3, 21), (3, 24), (3, 24), (3, 24),
    (3, 27), (3, 30), (3, 57),
]
# input plane range (inclusive) per conv; plane 0 = x, plane i = x_i
_IN_RANGES = {
    1: (0, 0), 2: (1, 1), 3: (1, 2), 4: (2, 3), 5: (3, 4), 6: (3, 5),
    7: (3, 6), 8: (4, 7), 9: (4, 8), 10: (5, 9), 11: (5, 10), 12: (6, 11),
    13: (6, 12), 14: (7, 13), 15: (7, 14), 16: (8, 15), 17: (9, 16),
    18: (9, 17), 19: (9, 18), 20: (1, 19),
}

H = W = 512
R = 36                 # output rows per block
NBLK = 15              # 14 full blocks + 1 ragged (8 rows)
LASTROWS = H - 14 * R  # 8
HALO = 3               # row halo stored in shingle window
WIN = R + 2 * HALO     # 42
PADW = 3
PW = PADW + W + PADW   # 518
PH = HALO + H + 33     # 548 plane rows (3 top pad, 512 data, 33 bottom pad)
NPLANES = 20
NCORES = 8

M = 3 * R              # 108 matmul output partitions
KP = 3 * WIN           # 126 contraction partitions per triple

_NMAT = [k * (cin // 3) for (k, cin) in _CONV_SPECS]  # matmuls per block per conv
_MAT_OFF = np.concatenate([[0], np.cumsum(_NMAT)]).astype(int)  # lhsT col-block offsets
_TOTAL_MAT = int(_MAT_OFF[-1])  # 403

_cache = {}


def _make_lhsT_all(params):
    """[126, total_mats*108] f32; per conv j blocks ordered (t, dx)."""
    out = np.zeros((KP, _TOTAL_MAT * M), np.float32)
    ys = np.arange(R)
    for j in range(1, 21):
        k, cin = _CONV_SPECS[j - 1]
        p = (k - 1) // 2
        w = np.asarray(params[f"w{j}"], np.float32)  # (3, cin, k, k)
        T = cin // 3
        for t in range(T):
            for dx in range(k):
                mat = np.zeros((3, WIN, 3, R), np.float32)  # (ci, d, co, y)
                for dy in range(k):
                    ds = ys + HALO + dy - p  # 0 <= ds < WIN
                    wt = w[:, 3 * t:3 * t + 3, dy, dx]  # (co, ci)
                    mat[:, ds, :, ys] = np.broadcast_to(wt.T, (R, 3, 3))
                mi = _MAT_OFF[j - 1] + t * k + dx
                out[:, mi * M:(mi + 1) * M] = mat.reshape(KP, M)
    return out


def _make_bias_all(params):
    out = np.zeros((M, 20), np.float32)
    for j in range(1, 21):
        out[:, j - 1] = np.repeat(np.asarray(params[f"b{j}"], np.float32), R)
    return out


def _build_program(n_rep=1):
    nc = bass.Bass()
    x_in = nc.declare_dram_parameter("x", [3, H, W], mybir.dt.float32, isOutput=False)
    lhsT_in = nc.declare_dram_parameter(
        "lhsT", [KP, _TOTAL_MAT * M], mybir.dt.bfloat16, isOutput=False)
    bias_in = nc.declare_dram_parameter("bias", [M, 20], mybir.dt.float32, isOutput=False)
    out_t = nc.declare_dram_parameter("out", [3, H, W], mybir.dt.float32, isOutput=True)
    planes = nc.dram_tensor("planes", [NPLANES, 3, PH, PW], mybir.dt.bfloat16)

    TMAX = max(cin // 3 for (_, cin) in _CONV_SPECS)  # 19
    LW_MAX = max(_NMAT)  # 57 matrices for conv20

    events = [(j, b) for j in range(1, 21) for b in range(NBLK)]
    ncb = len(events)

    def cbi(j, b):
        return (j - 1) * NBLK + b

    # lhsT load schedule: conv j -> slot j % 2; load gen per slot
    lhsT_gen = {}  # j -> (slot, gen)
    slot_loads = [0, 0]
    for j in range(1, 21):
        s = j % 2
        slot_loads[s] += 1
        lhsT_gen[j] = (s, slot_loads[s])

    # ---- python-side semaphore accounting ----
    # evac DMA map: cb -> (staging slot, list of DMA issue indices)
    # staging ring: 4 slots; slot of cb = cb % 4
    # shingle ring: 3 slots; slot of cb = cb % 3
    # per-slot cumulative inc counts for exact thresholds
    shg_inc = [0, 0, 0]          # s_shg[slot] totals (16 per DMA)
    shg_ready_at = {}            # cb -> (slot, threshold)
    slotfree_needed = {}         # cb(shingle dma) -> (slot, threshold on s_slotfree)
    stag_inc = [0, 0, 0, 0]      # s_stagfree[slot] totals
    stag_ready_at = {}           # cb -> (slot, threshold) evac-DMA(cb) completion
    xblk_inc = [0, 0]
    outfree_inc = [0, 0]

    from contextlib import ExitStack

    with ExitStack() as ctx:
        def sbuf(name, shape, dt):
            return ctx.enter_context(nc.sbuf_tensor(name, shape, dt))

        def sem(name):
            return ctx.enter_context(nc.semaphore(name))

        bf = mybir.dt.bfloat16
        f32 = mybir.dt.float32
        zt = sbuf("zt", [128, H * PADW], bf)
        sh = [sbuf(f"sh{i}", [KP, TMAX * PW], bf) for i in range(3)]
        lw = [sbuf(f"lw{i}", [KP, LW_MAX * M], bf) for i in range(2)]
        bias_sb = sbuf("bias_sb", [M, 20], f32)
        st = [sbuf(f"st{i}", [M, W], bf) for i in range(4)]
        xb = [sbuf(f"xb{i}", [M, W], bf) for i in range(2)]
        t1 = [sbuf(f"t1{i}", [M, W], bf) for i in range(2)]
        t2 = [sbuf(f"t2{i}", [M, W], bf) for i in range(2)]
        osb = [sbuf(f"osb{i}", [M, W], f32) for i in range(2)]
        ps = [ctx.enter_context(nc.psum_tensor(f"ps{i}", [M, W], f32))
              for i in range(2)]
        s_zt = sem("s_zt")
        s_init = sem("s_init")
        s_lhsT = [sem(f"s_lhsT{i}") for i in range(2)]
        s_shg = [sem(f"s_shg{i}") for i in range(3)]
        s_sf = [sem(f"s_sf{i}") for i in range(3)]
        s_pe = sem("s_pe")
        s_act = sem("s_act")
        s_layer = sem("s_layer")
        s_stg = [sem(f"s_stg{i}") for i in range(4)]
        s_xblk = [sem(f"s_xblk{i}") for i in range(2)]
        s_xf = [sem(f"s_xf{i}") for i in range(2)]
        s_dve = sem("s_dve")
        s_aout = sem("s_aout")
        s_of = [sem(f"s_of{i}") for i in range(2)]
        block = ctx.enter_context(nc.Block())

        # ---------- precompute schedules (pure python) ----------
        # shingle DMAs for cb use slot cb % 3 (one DMA per input triple).
        for cb, (j, b) in enumerate(events):
            s = cb % 3
            a, bmax = _IN_RANGES[j]
            shg_inc[s] += 16 * (bmax - a + 1)
            shg_ready_at[cb] = (s, shg_inc[s])
            slotfree_needed[cb] = (s, cb // 3)  # uses of the slot before this one
        # evac DMAs: cb (j<=19) -> slot cb % 4; last block emits 3 DMAs.
        for cb, (j, b) in enumerate(events):
            if j > 19:
                continue
            s = cb % 4
            stag_inc[s] += 16 * (3 if b == NBLK - 1 else 1)
            stag_ready_at[cb] = (s, stag_inc[s])
        # conv20 staging slots are freed by DVE (inc 16 per use)
        stg20_free_at = {}
        stag_inc2 = list(stag_inc)
        for b in range(NBLK):
            cb = cbi(20, b)
            s = cb % 4
            stag_inc2[s] += 16
            stg20_free_at[cb] = (s, stag_inc2[s])

        # staging slot "gen" counters for reuse waits (ACT side):
        # cumulative inc value of s_stg[slot] BEFORE the current use must be
        # reached for the slot to be free.
        stg_prev_thresh = {}
        run_inc = [0, 0, 0, 0]
        for cb, (j, b) in enumerate(events):
            s = cb % 4
            stg_prev_thresh[cb] = run_inc[s]
            if j <= 19:
                run_inc[s] += 16 * (3 if b == NBLK - 1 else 1)
            else:
                run_inc[s] += 16  # freed by DVE

        # shingle input-readiness: needed evac completions per shingle(j, bs)
        def needed_evacs(j, bs):
            a, bmax = _IN_RANGES[j]
            need = []
            for i in range(max(a, 1), bmax + 1):
                for beta in (bs - 1, bs, bs + 1):
                    if 0 <= beta <= NBLK - 1:
                        need.append(cbi(i, beta))
            return need

        shingle_waits = {}  # cb -> list of (slot, threshold) on s_stg
        for cb, (j, b) in enumerate(events):
            per_slot = {}
            for pcb in needed_evacs(j, b):
                s, thr = stag_ready_at[pcb]
                per_slot[s] = max(per_slot.get(s, 0), thr)
            shingle_waits[cb] = sorted(per_slot.items())

        # init DMA count for s_init (16 per DMA): 80 pad zeros + bias + 2 lhsT
        # (on SP) + 1 x-cast (gpsimd)
        N_INIT_DMAS = NPLANES * 4 + 1 + 1
        INIT_TARGET = 16 * N_INIT_DMAS

        # xblk / outfree gen tracking
        xblk_thresh = {}
        for b in range(NBLK):
            s = b % 2
            xblk_inc[s] += 16
            xblk_thresh[b] = xblk_inc[s]
        outfree_thresh = {}
        for b in range(NBLK):
            s = b % 2
            outfree_thresh[b] = outfree_inc[s]  # value before this use
            outfree_inc[s] += 16 * (3 if b == NBLK - 1 else 1)

        def lhsT_thresh(j):
            return 16 * lhsT_gen[j][1]

        # per-repetition totals for all monotonic semaphores
        TOT_shg = list(shg_inc)
        TOT_sf = [sum(1 for cb in range(ncb) if cb % 3 == s2) for s2 in range(3)]
        TOT_stg = list(run_inc)
        TOT_lhsT = [16 * slot_loads[0], 16 * slot_loads[1]]
        TOT_xblk = list(xblk_inc)
        TOT_xf = [sum(1 for b2 in range(NBLK) if b2 % 2 == s2) for s2 in range(2)]
        TOT_of = list(outfree_inc)

        # ---------------- GPSIMD: zero tile + x cast ----------------
        @block.gpsimd
        def _(gp):
            gp.memset(zt[:], 0.0).then_inc(s_zt, 1)
            gp.dma_start(
                out=planes[0, :, HALO:HALO + H, PADW:PADW + W], in_=x_in[:, :, :]
            ).then_inc(s_init, 16)

        # ---------------- SP: all HWDGE DMAs ----------------
        @block.sync
        def _(sync):
            evacs_done = 0  # bookkeeping only

            # init: weights + bias first (no zt dependency)
            j1s, j1g = lhsT_gen[1]
            sync.dma_start(
                out=lw[j1s][:, :_NMAT[0] * M],
                in_=lhsT_in[:, _MAT_OFF[0] * M:_MAT_OFF[1] * M],
            ).then_inc(s_lhsT[j1s], 16)
            j2s, j2g = lhsT_gen[2]
            sync.dma_start(
                out=lw[j2s][:, :_NMAT[1] * M],
                in_=lhsT_in[:, _MAT_OFF[1] * M:_MAT_OFF[2] * M],
            ).then_inc(s_lhsT[j2s], 16)
            sync.dma_start(out=bias_sb[:], in_=bias_in[:]).then_inc(s_init, 16)
            # plane pad zeroing
            sync.wait_ge(s_zt, 1)
            for i in range(NPLANES):
                sync.dma_start(out=planes[i, :, 0:HALO, :], in_=zt[0:9, 0:PW]
                               ).then_inc(s_init, 16)
                sync.dma_start(out=planes[i, :, HALO + H:PH, :], in_=zt[0:99, 0:PW]
                               ).then_inc(s_init, 16)
                zsrc = zt[0:3, 0:H * PADW].rearrange("p (r w) -> p r w", w=PADW)
                sync.dma_start(out=planes[i, :, HALO:HALO + H, 0:PADW],
                               in_=zsrc).then_inc(s_init, 16)
                sync.dma_start(out=planes[i, :, HALO:HALO + H, PADW + W:PW],
                               in_=zsrc).then_inc(s_init, 16)
            sync.wait_ge(s_init, INIT_TARGET)

            def issue_shingle(rep, cb2):
                j2, b2 = events[cb2]
                a, bmax = _IN_RANGES[j2]
                T = bmax - a + 1
                s, free_thr = slotfree_needed[cb2]
                free_thr += rep * TOT_sf[s]
                if free_thr > 0:
                    sync.wait_ge(s_sf[s], free_thr)
                for ss, thr in shingle_waits[cb2]:
                    sync.wait_ge(s_stg[ss], thr + rep * TOT_stg[ss])
                for t in range(T):
                    sync.dma_start(
                        out=sh[s][:, t * PW:(t + 1) * PW],
                        in_=planes[a + t, :, R * b2:R * b2 + WIN, :],
                    ).then_inc(s_shg[s], 16)

            def issue_evac(rep, pcb):
                pj, pb = events[pcb]
                s = pcb % 4
                sync.wait_ge(s_act, rep * ncb + pcb + 1)
                if pb < NBLK - 1:
                    sync.dma_start(
                        out=planes[pj, :, HALO + R * pb:HALO + R * pb + R,
                                   PADW:PADW + W],
                        in_=st[s][:],
                    ).then_inc(s_stg[s], 16)
                else:
                    for c in range(3):
                        sync.dma_start(
                            out=planes[pj, c, HALO + R * pb:HALO + R * pb + LASTROWS,
                                       PADW:PADW + W],
                            in_=st[s][c * R:c * R + LASTROWS, :],
                        ).then_inc(s_stg[s], 16)

            for rep in range(n_rep):
                if rep > 0:
                    # reload conv1/conv2 weights (slots recycled by later layers)
                    sync.wait_ge(s_layer, (rep - 1) * 20 + 19)
                    sync.dma_start(
                        out=lw[lhsT_gen[1][0]][:, :_NMAT[0] * M],
                        in_=lhsT_in[:, _MAT_OFF[0] * M:_MAT_OFF[1] * M],
                    ).then_inc(s_lhsT[lhsT_gen[1][0]], 16)
                    sync.wait_ge(s_layer, rep * 20)
                    sync.dma_start(
                        out=lw[lhsT_gen[2][0]][:, :_NMAT[1] * M],
                        in_=lhsT_in[:, _MAT_OFF[1] * M:_MAT_OFF[2] * M],
                    ).then_inc(s_lhsT[lhsT_gen[2][0]], 16)
                # prologue: first two shingles
                issue_shingle(rep, cbi(1, 0))
                issue_shingle(rep, cbi(1, 1))

                for cb, (j, b) in enumerate(events):
                    if b == 0:
                        if 2 <= j <= 20 and (j - 1) <= 19:
                            issue_evac(rep, cbi(j - 1, NBLK - 1))
                    else:
                        if j <= 19:
                            issue_evac(rep, cbi(j, b - 1))
                    # lhsT prefetch for conv j+1 early in layer j
                    if b == 2 and j >= 2 and j + 1 <= 20:
                        jn = j + 1
                        s2, g2 = lhsT_gen[jn]
                        sync.wait_ge(s_layer, rep * 20 + j - 1)
                        sync.dma_start(
                            out=lw[s2][:, :_NMAT[jn - 1] * M],
                            in_=lhsT_in[:, _MAT_OFF[jn - 1] * M:_MAT_OFF[jn] * M],
                        ).then_inc(s_lhsT[s2], 16)
                    # next shingle (cb+2)
                    if cb + 2 < ncb:
                        issue_shingle(rep, cb + 2)
                    elif rep + 1 < n_rep:
                        pass  # next rep's prologue handles the first shingles
                    # conv20 pointwise data movement
                    if j == 20:
                        if b == 0:
                            for bb in (0, 1):
                                sync.wait_ge(
                                    s_xf[bb % 2], rep * TOT_xf[bb % 2])
                                sync.dma_start(
                                    out=xb[bb % 2][:],
                                    in_=planes[0, :, HALO + R * bb:HALO + R * bb + R,
                                               PADW:PADW + W],
                                ).then_inc(s_xblk[bb % 2], 16)
                        if 1 <= b <= NBLK - 2:
                            bb = b + 1
                            s2 = bb % 2
                            sync.wait_ge(s_xf[s2], rep * TOT_xf[s2] + bb // 2)
                            sync.dma_start(
                                out=xb[s2][:],
                                in_=planes[0, :, HALO + R * bb:HALO + R * bb + R,
                                           PADW:PADW + W],
                            ).then_inc(s_xblk[s2], 16)
                        if b >= 1:
                            bo = b - 1
                            sync.wait_ge(s_aout, rep * NBLK + bo + 1)
                            sync.dma_start(
                                out=out_t[:, R * bo:R * bo + R, :],
                                in_=osb[bo % 2][:],
                            ).then_inc(s_of[bo % 2], 16)
                # tail: out store for block 14
                bo = NBLK - 1
                sync.wait_ge(s_aout, rep * NBLK + bo + 1)
                for c in range(3):
                    sync.dma_start(
                        out=out_t[c, R * bo:R * bo + LASTROWS, :],
                        in_=osb[bo % 2][c * R:c * R + LASTROWS, :],
                    ).then_inc(s_of[bo % 2], 16)

        # ---------------- PE: matmuls ----------------
        @block.tensor
        def _(pe):
            for rep in range(n_rep):
                for cb, (j, b) in enumerate(events):
                    gcb = rep * ncb + cb
                    k, cin = _CONV_SPECS[j - 1]
                    p = (k - 1) // 2
                    T = cin // 3
                    s, thr = shg_ready_at[cb]
                    pe.wait_ge(s_shg[s], thr + rep * TOT_shg[s])
                    if gcb >= 2:
                        pe.wait_ge(s_act, gcb - 1)  # psum[gcb%2] free
                    if b == 0:
                        ls, _g = lhsT_gen[j]
                        pe.wait_ge(s_lhsT[ls],
                                   lhsT_thresh(j) + rep * TOT_lhsT[ls])
                    ls = lhsT_gen[j][0]
                    nmat = T * k
                    mi = 0
                    mm = None
                    for t in range(T):
                        base_col = t * PW + PADW - p
                        for dx in range(k):
                            mm = pe.matmul(
                                ps[gcb % 2][:],
                                lw[ls][:, (t * k + dx) * M:(t * k + dx + 1) * M],
                                sh[s][:, base_col + dx:base_col + dx + W],
                                start=(mi == 0),
                                stop=(mi == nmat - 1),
                            )
                            mi += 1
                    mm.then_inc(s_pe, 1)
                    pe.nop().then_inc(s_sf[s], 1)
                    if b == NBLK - 1:
                        pe.nop().then_inc(s_layer, 1)

        # ---------------- ACT: bias+relu evac + final relu ----------------
        @block.scalar
        def _(act):
            def final_act(rep, bo):
                act.wait_ge(s_dve, rep * NBLK + bo + 1)
                of_thr = outfree_thresh[bo] + rep * TOT_of[bo % 2]
                if of_thr > 0:
                    act.wait_ge(s_of[bo % 2], of_thr)
                act.activation(
                    osb[bo % 2][:], t2[bo % 2][:],
                    mybir.ActivationFunctionType.Relu, bias=1.0, scale=1.0,
                ).then_inc(s_aout, 1)

            for rep in range(n_rep):
                for cb, (j, b) in enumerate(events):
                    gcb = rep * ncb + cb
                    act.wait_ge(s_pe, gcb + 1)
                    s = cb % 4
                    stg_thr = stg_prev_thresh[cb] + rep * TOT_stg[s]
                    if stg_thr > 0:
                        act.wait_ge(s_stg[s], stg_thr)
                    act.activation(
                        st[s][:], ps[gcb % 2][:], mybir.ActivationFunctionType.Relu,
                        bias=bias_sb[:, j - 1:j], scale=1.0,
                    ).then_inc(s_act, 1)
                    if j == 20 and b >= 1:
                        final_act(rep, b - 1)
                final_act(rep, NBLK - 1)

        # ---------------- DVE: pointwise x20*x - x20 ----------------
        @block.vector
        def _(dve):
            for rep in range(n_rep):
                for b in range(NBLK):
                    cb = cbi(20, b)
                    gcb = rep * ncb + cb
                    s = cb % 4
                    dve.wait_ge(s_act, gcb + 1)
                    dve.wait_ge(s_xblk[b % 2],
                                xblk_thresh[b] + rep * TOT_xblk[b % 2])
                    gb = rep * NBLK + b
                    if gb >= 2:
                        dve.wait_ge(s_aout, gb - 1)  # t2[b%2] free
                    dve.tensor_tensor(
                        t1[b % 2][:], st[s][:], xb[b % 2][:], mybir.AluOpType.mult)
                    dve.tensor_tensor(
                        t2[b % 2][:], t1[b % 2][:], st[s][:],
                        mybir.AluOpType.subtract,
                    ).then_inc(s_dve, 1)
                    dve.nop().then_inc(s_stg[s], 16)
                    dve.nop().then_inc(s_xf[b % 2], 1)

    return nc


def _get_program(n_rep=1):
    key = ("nc", n_rep)
    if key not in _cache:
        _cache[key] = _build_program(n_rep)
    return _cache[key]


def kernel(x, params):
    x = np.asarray(x, np.float32)
    assert x.shape == (NCORES, 3, H, W)
    lhsT_all = _make_lhsT_all(params).astype(_BF16)
    bias_all = _make_bias_all(params)
    nc = _get_program()
    in_maps = [
        {"x": np.ascontiguousarray(x[i]), "lhsT": lhsT_all, "bias": bias_all}
        for i in range(NCORES)
    ]
    res = run_bass_kernel_spmd(nc, in_maps, core_ids=list(range(NCORES)))
    return np.stack([res.results[i]["out"] for i in range(NCORES)], axis=0)
